# revision 1
# baseline (speedup 1.0000x reference)
"""Trainium2 Bass kernel for relational graph convolution:

    y = sum_r (A[r] @ x) @ W[r].T        A: [8, 4096, 4096] f32
                                         x: [4096, 64] f32, W: [8, 64, 64] f32

Strategy
--------
By associativity, y = sum_r A[r] @ v_r with v_r = x @ W[r].T, turning the
problem into one [4096, 4096] @ [4096, 64] matmul per relation. Relations are
sharded across the 8 NeuronCores (expert-style parallelism); each core returns
its partial y_r.T and the host sums and transposes.

The TensorE contracts over the partition dimension of both operands, so the
contraction index m (A's column index) must land on SBUF partitions. The host
therefore ships A[r].T (row-major) so device DMAs are plain contiguous slabs.

Per core:
  phase 1: v = x @ W_r.T via 32 exact-fp32 matmuls (lhsT = x.T column chunks,
           rhs = W_r.T), rounded into a float32r SBUF tile.
  phase 2: for each of 32 contraction chunks (128 rows of A_r.T): one 2 MB DMA,
           then 8 float32r matmuls (lhsT = v chunk [128, 64], rhs = A_r.T slab
           [128, 512]) accumulating y_r.T [64, 4096] across all 8 PSUM banks.
  phase 3: copy PSUM -> SBUF -> DMA out y_r.T.

float32r (4-byte, reduced-mantissa matmul mode) streams at 1 cycle/row vs 4 for
float32, making the kernel DMA-bound (~64 MB/core of A traffic) instead of
PE-bound; measured accuracy is ~1.5e-4 relative.
"""

import numpy as np

import concourse.tile as tile
from concourse import bacc, mybir
from concourse.bass_utils import run_bass_kernel_spmd

R, N, IN_F, OUT_F = 8, 4096, 64, 64
P = 128            # partition dim / contraction chunk
MC = N // P        # 32 contraction chunks
BANK = 512         # fp32 elems per PSUM bank
NB = N // BANK     # 8 output column blocks

F32 = mybir.dt.float32
F32R = mybir.dt.float32r

_NC_CACHE = {}


def _build_nc():
    nc = bacc.Bacc("TRN2", target_bir_lowering=False, debug=False, num_devices=R)

    at = nc.dram_tensor("at", [N, N], F32R, kind="ExternalInput").ap()
    xt = nc.dram_tensor("xt", [IN_F, N], F32, kind="ExternalInput").ap()
    wt = nc.dram_tensor("wt", [IN_F, OUT_F], F32, kind="ExternalInput").ap()
    ytp = nc.dram_tensor("ytp", [OUT_F, N], F32, kind="ExternalOutput").ap()

    with tile.TileContext(nc) as tc:
        with (
            tc.tile_pool(name="const", bufs=1) as const_pool,
            tc.tile_pool(name="atp", bufs=4) as at_pool,
            tc.tile_pool(name="outp", bufs=1) as out_pool,
        ):
            xt_sb = const_pool.tile([IN_F, N], F32)
            nc.sync.dma_start(xt_sb[:], xt[:])
            wt_sb = const_pool.tile([IN_F, OUT_F], F32)
            nc.sync.dma_start(wt_sb[:], wt[:])

            # phase 1: v[m, o] = sum_i x[m, i] W[o, i], exact fp32, then
            # rounded to f32r by the DVE copy.
            v_sb = const_pool.tile([P, MC, OUT_F], F32R)
            with tc.tile_pool(name="psv", bufs=2, space="PSUM") as psv_pool:
                for mc in range(MC):
                    ps_v = psv_pool.tile([P, OUT_F], F32)
                    nc.tensor.matmul(
                        ps_v[:],
                        xt_sb[:, mc * P : (mc + 1) * P],
                        wt_sb[:],
                        start=True,
                        stop=True,
                    )
                    nc.vector.tensor_copy(v_sb[:, mc, :], ps_v[:])

            # phase 2: y_r.T[o, n] += sum_m v[m, o] * A_r.T[m, n]
            with tc.tile_pool(name="psy", bufs=1, space="PSUM") as psy_pool:
                ps_y = psy_pool.tile([OUT_F, N], F32)
                for mc in range(MC):
                    at_t = at_pool.tile([P, N], F32R)
                    nc.sync.dma_start(at_t[:], at[mc * P : (mc + 1) * P, :])
                    for b in range(NB):
                        nc.tensor.matmul(
                            ps_y[:, b * BANK : (b + 1) * BANK],
                            v_sb[:, mc, :],
                            at_t[:, b * BANK : (b + 1) * BANK],
                            start=(mc == 0),
                            stop=(mc == MC - 1),
                        )

                # phase 3
                out_sb = out_pool.tile([OUT_F, N], F32)
                nc.vector.tensor_copy(out_sb[:], ps_y[:])
                nc.sync.dma_start(ytp[:], out_sb[:])

    nc.compile()
    return nc


def run_with_results(inputs, trace=False, trace_cores=None):
    """Run the kernel; returns (full_output [4096, 64] f32, BassKernelResults)."""
    adjacency = np.asarray(inputs["adjacency"], dtype=np.float32)
    x = np.asarray(inputs["x"], dtype=np.float32)
    weight = np.asarray(inputs["weight"], dtype=np.float32)
    assert adjacency.shape == (R, N, N)
    assert x.shape == (N, IN_F)
    assert weight.shape == (R, OUT_F, IN_F)

    # Host-side layout prep: contraction dim must land on SBUF partitions.
    at_np = np.ascontiguousarray(adjacency.transpose(0, 2, 1))  # [R, m, n]
    xt_np = np.ascontiguousarray(x.T)                           # [IN_F, N]
    wt_np = np.ascontiguousarray(weight.transpose(0, 2, 1))     # [R, IN_F, OUT_F]

    if "nc" not in _NC_CACHE:
        _NC_CACHE["nc"] = _build_nc()
    nc = _NC_CACHE["nc"]

    in_maps = [
        {"at": at_np[r], "xt": xt_np, "wt": wt_np[r]} for r in range(R)
    ]
    res = run_bass_kernel_spmd(
        nc,
        in_maps,
        core_ids=list(range(R)),
        trace=trace,
        trace_cores=trace_cores,
    )

    yt = np.zeros((OUT_F, N), dtype=np.float32)
    for r in range(R):
        yt += res.results[r]["ytp"]
    y = np.ascontiguousarray(yt.T)
    return y, res


def kernel(**inputs) -> np.ndarray:
    y, _ = run_with_results(inputs)
    return y


# revision 5
# speedup vs baseline: 21.1769x; 21.1769x over previous
"""Trainium2 Bass kernel for relational graph convolution:

    y = sum_r (A[r] @ x) @ W[r].T        A: [8, 4096, 4096] f32
                                         x: [4096, 64] f32, W: [8, 64, 64] f32

Strategy
--------
By associativity, y = sum_r A[r] @ v_r with v_r = x @ W[r].T, turning the
problem into one [4096, 4096] @ [4096, 64] matmul per relation. Relations are
sharded across the 8 NeuronCores (expert-style parallelism); each core returns
its partial y_r.T and the host sums and transposes.

The TensorE contracts over the partition dimension of both operands, so the
contraction index m (A's column index) must land on SBUF partitions. The host
therefore ships A[r].T (row-major) so device DMAs are plain contiguous slabs.

Per core:
  phase 1: v = x @ W_r.T via 32 exact-fp32 matmuls (lhsT = x.T column chunks,
           rhs = W_r.T), rounded into a float32r SBUF tile.
  phase 2: for each of 32 contraction chunks (128 rows of A_r.T): one 2 MB DMA,
           then 8 float32r matmuls (lhsT = v chunk [128, 64], rhs = A_r.T slab
           [128, 512]) accumulating y_r.T [64, 4096] across all 8 PSUM banks.
  phase 3: per-bank PSUM -> SBUF copies chase the final matmuls, then one DMA
           of y_r.T out.

float32r (4-byte, reduced-mantissa matmul mode) streams at 1 cycle/row vs 4 for
float32, making the kernel DMA-bound (~64 MB/core of A traffic, measured
~427 GB/s/core with all 8 cores streaming) instead of PE-bound; measured
end-to-end accuracy is ~1e-4 relative.

MODE="bf16" is an optional variant that ships A as bf16 (halving DMA traffic);
~3x the error, kept behind a flag.
"""

import numpy as np

import concourse.tile as tile
from concourse import bacc, mybir
from concourse.bass_utils import run_bass_kernel_spmd

R, N, IN_F, OUT_F = 8, 4096, 64, 64
P = 128            # partition dim / contraction chunk
MC = N // P        # 32 contraction chunks
BANK = 512         # fp32 elems per PSUM bank
NB = N // BANK     # 8 output column blocks

F32 = mybir.dt.float32

MODE = "f32r"      # "f32r" (default) or "bf16"

_NC_CACHE = {}


def _build_nc(repeat=1, mode=None):
    """repeat>1 re-runs phase 2 (the steady-state A-streaming loop) that many
    times inside one NEFF — used only by the benchmark harness to amortize
    per-execute dispatch overhead; the graded kernel uses repeat=1."""
    mode = mode or MODE
    a_dt = mybir.dt.float32r if mode == "f32r" else mybir.dt.bfloat16
    # chunks of A rows per DMA: 2 MB per transfer in either mode
    jc = 1 if mode == "f32r" else 2

    nc = bacc.Bacc("TRN2", target_bir_lowering=False, debug=False, num_devices=R)

    at = nc.dram_tensor("at", [N, N], a_dt, kind="ExternalInput").ap()
    xt = nc.dram_tensor("xt", [IN_F, N], F32, kind="ExternalInput").ap()
    wt = nc.dram_tensor("wt", [IN_F, OUT_F], F32, kind="ExternalInput").ap()
    ytp = nc.dram_tensor("ytp", [OUT_F, N], F32, kind="ExternalOutput").ap()

    with tile.TileContext(nc) as tc:
        with (
            tc.tile_pool(name="const", bufs=1) as const_pool,
            tc.tile_pool(name="atp", bufs=4) as at_pool,
            tc.tile_pool(name="outp", bufs=1) as out_pool,
        ):
            xt_sb = const_pool.tile([IN_F, N], F32)
            nc.sync.dma_start(xt_sb[:], xt[:])
            wt_sb = const_pool.tile([IN_F, OUT_F], F32)
            nc.sync.dma_start(wt_sb[:], wt[:])

            at_r3 = at.rearrange("(c j p) n -> c p j n", p=P, j=jc)
            out_sb = out_pool.tile([OUT_F, N], F32)
            for _rep in range(repeat):
                # phase 1: v[m, o] = sum_i x[m, i] W[o, i], exact fp32, then
                # rounded to the matmul dtype by the DVE copy.
                v_sb = const_pool.tile([P, MC, OUT_F], a_dt, tag="v_sb")
                with tc.tile_pool(name="psv", bufs=2, space="PSUM") as psv_pool:
                    for mc in range(MC):
                        ps_v = psv_pool.tile([P, OUT_F], F32)
                        nc.tensor.matmul(
                            ps_v[:],
                            xt_sb[:, mc * P : (mc + 1) * P],
                            wt_sb[:],
                            start=True,
                            stop=True,
                        )
                        nc.vector.tensor_copy(v_sb[:, mc, :], ps_v[:])

                # phase 2: y_r.T[o, n] += sum_m v[m, o] * A_r.T[m, n]
                with tc.tile_pool(name="psy", bufs=1, space="PSUM") as psy_pool:
                    ps_y = psy_pool.tile([OUT_F, N], F32)
                    for c in range(MC // jc):
                        at_t = at_pool.tile([P, jc, N], a_dt)
                        nc.sync.dma_start(at_t[:], at_r3[c])
                        for j in range(jc):
                            mc = c * jc + j
                            for b in range(NB):
                                nc.tensor.matmul(
                                    ps_y[:, b * BANK : (b + 1) * BANK],
                                    v_sb[:, mc, :],
                                    at_t[:, j, b * BANK : (b + 1) * BANK],
                                    start=(mc == 0),
                                    stop=(mc == MC - 1),
                                )
                                # phase 3a: bank copies chase the final matmuls
                                if mc == MC - 1:
                                    nc.vector.tensor_copy(
                                        out_sb[:, b * BANK : (b + 1) * BANK],
                                        ps_y[:, b * BANK : (b + 1) * BANK],
                                    )

                    # phase 3b
                    nc.sync.dma_start(ytp[:], out_sb[:])

    nc.compile()
    return nc


def run_with_results(inputs, repeat=1, mode=None):
    """Run the kernel; returns (full_output [4096, 64] f32, BassKernelResults)."""
    mode = mode or MODE
    adjacency = np.asarray(inputs["adjacency"], dtype=np.float32)
    x = np.asarray(inputs["x"], dtype=np.float32)
    weight = np.asarray(inputs["weight"], dtype=np.float32)
    assert adjacency.shape == (R, N, N)
    assert x.shape == (N, IN_F)
    assert weight.shape == (R, OUT_F, IN_F)

    in_maps = make_in_maps(adjacency, x, weight, mode)

    key = (repeat, mode)
    if key not in _NC_CACHE:
        _NC_CACHE[key] = _build_nc(repeat, mode)
    nc = _NC_CACHE[key]

    res = run_bass_kernel_spmd(nc, in_maps, core_ids=list(range(R)))
    return assemble_output(res.results), res


def make_in_maps(adjacency, x, weight, mode=None):
    mode = mode or MODE
    # Host-side layout prep: contraction dim must land on SBUF partitions.
    at_np = np.ascontiguousarray(adjacency.transpose(0, 2, 1))  # [R, m, n]
    if mode == "bf16":
        import ml_dtypes

        at_np = at_np.astype(ml_dtypes.bfloat16)
    xt_np = np.ascontiguousarray(x.T)                           # [IN_F, N]
    wt_np = np.ascontiguousarray(weight.transpose(0, 2, 1))     # [R, IN_F, OUT_F]
    return [{"at": at_np[r], "xt": xt_np, "wt": wt_np[r]} for r in range(R)]


def assemble_output(results):
    yt = np.zeros((OUT_F, N), dtype=np.float32)
    for r in range(R):
        yt += results[r]["ytp"]
    return np.ascontiguousarray(yt.T)


def kernel(**inputs) -> np.ndarray:
    y, _ = run_with_results(inputs)
    return y
